# revision 1
# baseline (speedup 1.0000x reference)
"""Trainium2 Bass kernel for nn_Complex_Only_46308337385506 (gnn_message_passing).

Math (derived + numerically validated against the jax reference):
  The per-edge orthonormal basis R (rows nU, nV, nJ) enters the output only
  through two per-edge scalars:
      gam = nJ_z = Jz/(|J|+eps)
      A1p = copysign(sqrt(Jx^2+Jy^2), gam+eps)/(|J|+eps)    (= -nU_z approx)
  With w = gam*Xz - A1p*Xx:
      Y0 = Wa@Xx + (Wa-Wc)@(A1p*w) + Wb@(gam*Xy)
      Y1 = Wa@Xy - Wb@(A1p*Xz + gam*Xx)
      Y2 = Wa@Xz + (Wc-Wa)@(gam*w) + Wb@(A1p*Xy)
  followed by the VN leaky-relu stage:
      d = Wd@Y (over channel dim), dot = <Y,d>_3, dn2 = <d,d>_3
      out = Y - 0.8*min(dot,0)/(dn2+eps) * d

Sharding: data-parallel over batch B=8 -> one batch per NeuronCore.
Per-core layout: supers of 1024 points; points are transposed on the PE
(pairs of feature blocks) so the E-contraction runs as [K<=128, N=512]
matmuls; stage-3 runs on [128, 512] tiles (two 512-pt groups stacked on
partitions).
"""

import math
import os
import numpy as np
from contextlib import ExitStack

import concourse.bass as bass
import concourse.bacc as bacc
import concourse.tile as tile
from concourse import mybir
from concourse import bass_utils

F32 = mybir.dt.float32
F32R = mybir.dt.float32r
U32 = mybir.dt.uint32
AF = mybir.ActivationFunctionType
ALU = mybir.AluOpType

EPS = 1e-6
NEG = 0.2

B, C, E = 8, 16384, 64
SUPER = 1024           # points per super-iteration
NSUP = C // SUPER      # 16
GROUP = 512            # matmul free dim (points)
NCHUNK = 8             # 128-pt chunks per super


_CUSTOM_OPS = {}


def _register_custom_dve_ops():
    """Register two fused DVE ops (module-level, idempotent):
      SQSUM_ANT: out = Src0^2 + Src1^2
      ADDSQ_ANT: out = Src0 + Src1^2
    Replaces {2x ACT Square + 1 DVE add} chains with one DVE pass each."""
    if _CUSTOM_OPS:
        return _CUSTOM_OPS
    import numpy as _np
    from concourse import dve_ops
    from concourse.dve_spec import Spec, Src0, Src1, lower, sq, _has_src1
    from concourse.dve_uop import DveOpSpec
    from concourse.dve_table_gen import dve_ver_for

    def make(name, body, ref):
        spec = Spec(body=body, reference=ref)
        opcode = dve_ops._CUSTOM_DVE_ROW_BASE + len(dve_ops.OPS)
        shas = {}
        for ver in ("v3", "v4"):
            try:
                s = DveOpSpec(name=name, opcode=opcode,
                              uops=lower(spec, ver=ver),
                              rd1_en=_has_src1(spec))
                shas[ver] = s.sha(ver)
            except Exception:
                pass
        op = dve_ops.DveOp(name, spec, subdim=False, uops_sha=shas)
        dve_ops.OPS.append(op)
        dve_ops.CUSTOM_DVE_SPECS[name] = spec
        dve_ops._SUB_OPCODE_FOR_NAME[name] = opcode
        assert opcode < 0x20
        return op

    _CUSTOM_OPS["SQSUM"] = make(
        "SQSUM_ANT", sq(Src0) + sq(Src1),
        lambda in0, in1, s0, s1, imm2:
            (in0.astype(_np.float32) * in0 + in1.astype(_np.float32) * in1))
    _CUSTOM_OPS["ADDSQ"] = make(
        "ADDSQ_ANT", Src0 + sq(Src1),
        lambda in0, in1, s0, s1, imm2:
            in0.astype(_np.float32) + in1.astype(_np.float32) * in1)
    return _CUSTOM_OPS


def _pin_act_table_set(arch: str):
    """Steer the ACT table-set chooser: all funcs this kernel uses must
    first-match natural_log_exp_and_others, so exactly one table load is
    emitted (the chooser first-matches in act_info.json order)."""
    from concourse import hw_specs
    tables = hw_specs.get_activation_tables(arch)  # cached dict, mutate in place
    mine = {AF.Ln, AF.Exp, AF.Square, AF.Copy, AF.Identity}
    for name, funcs in tables.items():
        if name != "natural_log_exp_and_others":
            funcs -= mine


def _build_nc():
    global OPS
    OPS = _register_custom_dve_ops()
    nc = bacc.Bacc("TRN2", debug=False)
    _pin_act_table_set(nc.m.arch)

    XS = nc.dram_tensor("XS", [C, 192], F32, kind="ExternalInput").ap()
    JS = nc.dram_tensor("JS", [C, 192], F32, kind="ExternalInput").ap()
    WMM = nc.dram_tensor("WMM", [6, 128, 128], F32, kind="ExternalInput").ap()
    OUT = nc.dram_tensor("OUT", [64, 3, C], F32, kind="ExternalOutput").ap()

    with tile.TileContext(nc) as tc, ExitStack() as ctx:
        const = ctx.enter_context(tc.tile_pool(name="const", bufs=1))
        io = ctx.enter_context(tc.tile_pool(name="io", bufs=2))
        sa = ctx.enter_context(tc.tile_pool(name="sa", bufs=1))
        prodp = ctx.enter_context(tc.tile_pool(name="prodp", bufs=2))
        rhsp = ctx.enter_context(tc.tile_pool(name="rhsp", bufs=2))
        xsbp = ctx.enter_context(tc.tile_pool(name="xsbp", bufs=2))
        s3p = ctx.enter_context(tc.tile_pool(name="s3p", bufs=1))
        outp = ctx.enter_context(tc.tile_pool(name="outp", bufs=2))
        psT = ctx.enter_context(tc.tile_pool(name="psT", bufs=1, space="PSUM"))
        psY = ctx.enter_context(tc.tile_pool(name="psY", bufs=1, space="PSUM"))
        psD = ctx.enter_context(tc.tile_pool(name="psD", bufs=2, space="PSUM"))

        # bias constants for ACT
        eps_c = const.tile([128, 1], F32, tag="eps_c")
        ln8_c = const.tile([128, 1], F32, tag="ln8_c")
        nc.gpsimd.memset(eps_c[:], EPS)
        nc.gpsimd.memset(ln8_c[:], float(math.log(1.0 - NEG)))
        sgn_c = const.tile([128, 1], U32, tag="sgn_c")
        nc.gpsimd.memset(sgn_c[:], 0x80000000)

        # weights + identity, loaded once
        wsb = const.tile([128, 6, 128], F32)
        nc.sync.dma_start(wsb[:], WMM.rearrange("n p m -> p n m"))
        LW_D = wsb[:, 4, :]      # blkdiag(WdT, WdT)
        IDT = wsb[:, 5, :]       # identity
        LW_A = wsb[:, 0, :]      # blkdiag(WaT, WaT)
        LW_2 = wsb[:, 1, :]      # blkdiag((Wa-Wc).T, (Wc-Wa).T)
        LW_B = wsb[:, 2, :]      # blkdiag(WbT, WbT)
        LW_1 = wsb[:, 3, 0:64]   # [WaT; -WbT], M=64

        X3 = XS.rearrange("(v u s p) w -> v p u s w", p=128, s=NCHUNK, u=2)
        J3 = JS.rearrange("(v u s p) w -> v p u s w", p=128, s=NCHUNK, u=2)

        for u in range(NSUP):
            if u % 2 == 0:
                xs2 = io.tile([128, 2, NCHUNK * 192], F32, tag="xs")
                js2 = io.tile([128, 2, NCHUNK * 192], F32, tag="js")
                nc.sync.dma_start(
                    xs2[:].rearrange("p u (s w) -> p u s w", s=NCHUNK,
                                     w=192), X3[u // 2])
                nc.sync.dma_start(
                    js2[:].rearrange("p u (s w) -> p u s w", s=NCHUNK,
                                     w=192), J3[u // 2])
            xs = xs2[:, u % 2]
            js = js2[:, u % 2]
            xv = xs.rearrange("p (s e c) -> p s e c", s=NCHUNK, e=E, c=3)
            jv = js.rearrange("p (s e c) -> p s e c", s=NCHUNK, e=E, c=3)

            def v3(t):  # [128, 512] tile -> [128, 8, 64] view
                return t[:].rearrange("p (s e) -> p s e", s=NCHUNK, e=E)

            # ---- stage A: per-edge scalars gam, A1p --------------------
            q = sa.tile([128, SUPER // 2], F32, tag="q")
            n2 = sa.tile([128, SUPER // 2], F32, tag="n2")
            nc.vector._custom_dve(OPS["SQSUM"], out=v3(q),
                                  in0=jv[:, :, :, 0], in1=jv[:, :, :, 1])
            nc.vector._custom_dve(OPS["ADDSQ"], out=v3(n2),
                                  in0=v3(q), in1=jv[:, :, :, 2])
            # ln-domain: t = rsqrt(n2) = exp(-0.5 ln n2);
            # |A1| = sqrt(q/n2) = exp(0.5 (ln q - ln n2)); sign from gam+eps.
            # (dropping the +EPS inside t shifts gam by ~1e-6 rel: negligible)
            lq = sa.tile([128, SUPER // 2], F32, tag="lq")
            ln2 = sa.tile([128, SUPER // 2], F32, tag="ln2")
            nc.scalar.activation(lq[:], q[:], AF.Ln)
            nc.scalar.activation(ln2[:], n2[:], AF.Ln)
            t_ = sa.tile([128, SUPER // 2], F32, tag="t_")
            nc.scalar.activation(t_[:], ln2[:], AF.Exp, scale=-0.5)
            df = sa.tile([128, SUPER // 2], F32, tag="df")
            nc.vector.tensor_tensor(df[:], lq[:], ln2[:], ALU.subtract)
            rho = sa.tile([128, SUPER // 2], F32, tag="rho")
            nc.scalar.activation(rho[:], df[:], AF.Exp, scale=0.5)
            gam = sa.tile([128, SUPER // 2], F32, tag="gam")
            nc.vector.tensor_tensor(v3(gam), jv[:, :, :, 2], v3(t_), ALU.mult)
            h = sa.tile([128, SUPER // 2], F32, tag="h")
            nc.vector.tensor_scalar(h[:], gam[:], EPS, None, ALU.add)
            a1 = sa.tile([128, SUPER // 2], F32, tag="a1")
            nc.vector.scalar_tensor_tensor(
                a1[:].bitcast(U32), h[:].bitcast(U32), sgn_c[:],
                rho[:].bitcast(U32), ALU.bitwise_and, ALU.bitwise_or)

            # ---- stage B: feature blocks into PROD slots ---------------
            # slots: 0=Xx 1=Xz 2=A1p*w 3=gam*w 4=gam*Xy 5=A1p*Xy 6=Xy 7=c8
            prod = prodp.tile([128, NCHUNK, 8, E], F32, tag="prod")
            nc.gpsimd.tensor_copy(prod[:, :, 0, :], xv[:, :, :, 0])
            nc.gpsimd.tensor_copy(prod[:, :, 1, :], xv[:, :, :, 2])
            nc.gpsimd.tensor_copy(prod[:, :, 6, :], xv[:, :, :, 1])
            m1 = sa.tile([128, SUPER // 2], F32, tag="m1")
            m2 = sa.tile([128, SUPER // 2], F32, tag="m2")
            wt = sa.tile([128, SUPER // 2], F32, tag="wt")
            nc.gpsimd.tensor_tensor(v3(m1), v3(gam), xv[:, :, :, 2], ALU.mult)
            nc.gpsimd.tensor_tensor(v3(m2), v3(a1), xv[:, :, :, 0], ALU.mult)
            nc.gpsimd.tensor_tensor(wt[:], m1[:], m2[:], ALU.subtract)
            nc.vector.tensor_tensor(prod[:, :, 2, :], v3(a1), v3(wt), ALU.mult)
            nc.vector.tensor_tensor(prod[:, :, 3, :], v3(gam), v3(wt), ALU.mult)
            nc.gpsimd.tensor_tensor(prod[:, :, 4, :], v3(gam), xv[:, :, :, 1],
                                    ALU.mult)
            nc.gpsimd.tensor_tensor(prod[:, :, 5, :], v3(a1), xv[:, :, :, 1],
                                    ALU.mult)
            m3 = sa.tile([128, SUPER // 2], F32, tag="m3")
            m4 = sa.tile([128, SUPER // 2], F32, tag="m4")
            nc.gpsimd.tensor_tensor(v3(m3), v3(a1), xv[:, :, :, 2], ALU.mult)
            nc.gpsimd.tensor_tensor(v3(m4), v3(gam), xv[:, :, :, 0], ALU.mult)
            nc.gpsimd.tensor_tensor(prod[:, :, 7, :], v3(m3), v3(m4), ALU.add)

            # ---- per group: transpose, matmuls, Y copies ---------------
            xsb = xsbp.tile([128, 3, GROUP], F32, tag="xsb")
            for g in range(2):
                tpX = psT.tile([128, GROUP], F32, tag="tpX")
                tp1 = psT.tile([128, GROUP], F32, tag="tp1")
                tp2 = psT.tile([128, GROUP], F32, tag="tp2")
                tp3 = psT.tile([128, GROUP], F32, tag="tp3")
                for k in range(4):
                    s = 4 * g + k
                    sl = slice(128 * k, 128 * (k + 1))
                    nc.tensor.transpose(tpX[:, sl], prod[:, s, 0:2, :], IDT)
                    nc.tensor.transpose(tp1[:, sl], prod[:, s, 2:4, :], IDT)
                    nc.tensor.transpose(tp2[:, sl], prod[:, s, 4:6, :], IDT)
                    nc.tensor.transpose(tp3[:, sl], prod[:, s, 6:8, :], IDT)
                rhX = rhsp.tile([128, GROUP], F32, tag="rhX")
                rh1 = rhsp.tile([128, GROUP], F32, tag="rh1")
                rh2 = rhsp.tile([128, GROUP], F32, tag="rh2")
                rh3 = rhsp.tile([128, GROUP], F32, tag="rh3")
                nc.scalar.activation(rhX[:], tpX[:], AF.Copy)
                nc.scalar.activation(rh1[:], tp1[:], AF.Copy)
                nc.scalar.activation(rh2[:], tp2[:], AF.Copy)
                nc.scalar.activation(rh3[:], tp3[:], AF.Copy)

                pA = psY.tile([128, GROUP], F32, tag="pA")
                pB = psY.tile([64, GROUP], F32, tag="pB")
                nc.tensor.matmul(pA[:], LW_A, rhX[:], start=True, stop=False)
                nc.tensor.matmul(pA[:], LW_2, rh1[:], start=False, stop=False)
                nc.tensor.matmul(pA[:], LW_B, rh2[:], start=False, stop=True)
                nc.tensor.matmul(pB[:], LW_1, rh3[:], start=True, stop=True)

                ro = slice(64 * g, 64 * (g + 1))
                nc.scalar.activation(xsb[ro, 0, :], pA[0:64, :], AF.Copy)
                nc.vector.tensor_copy(xsb[ro, 2, :], pA[64:128, :])
                nc.scalar.activation(xsb[ro, 1, :], pB[:], AF.Copy)

            # ---- Wd stage + VN leaky relu ------------------------------
            dsb = s3p.tile([128, 3, GROUP], F32, tag="dsb")
            for i in range(3):
                pd = psD.tile([128, GROUP], F32, tag="pd")
                nc.tensor.matmul(pd[:], LW_D, xsb[:, i, :], start=True,
                                 stop=True)
                nc.scalar.activation(dsb[:, i, :], pd[:], AF.Copy)

            xd0 = s3p.tile([128, GROUP], F32, tag="xd0")
            xd1 = s3p.tile([128, GROUP], F32, tag="xd1")
            xd2 = s3p.tile([128, GROUP], F32, tag="xd2")
            dot = s3p.tile([128, GROUP], F32, tag="dot")
            nc.gpsimd.tensor_tensor(xd0[:], xsb[:, 0, :], dsb[:, 0, :],
                                    ALU.mult)
            nc.gpsimd.tensor_tensor(xd1[:], xsb[:, 1, :], dsb[:, 1, :],
                                    ALU.mult)
            nc.gpsimd.tensor_tensor(xd2[:], xsb[:, 2, :], dsb[:, 2, :],
                                    ALU.mult)
            nc.vector.tensor_tensor(dot[:], xd0[:], xd1[:], ALU.add)
            nc.vector.tensor_tensor(dot[:], dot[:], xd2[:], ALU.add)

            dn2 = s3p.tile([128, GROUP], F32, tag="dn2")
            nc.vector._custom_dve(OPS["SQSUM"], out=dn2[:],
                                  in0=dsb[:, 0, :], in1=dsb[:, 1, :])
            nc.vector._custom_dve(OPS["ADDSQ"], out=dn2[:],
                                  in0=dn2[:], in1=dsb[:, 2, :])

            lnv = s3p.tile([128, GROUP], F32, tag="lnv")
            rec = s3p.tile([128, GROUP], F32, tag="rec")
            nc.scalar.activation(lnv[:], dn2[:], AF.Ln, bias=eps_c[:])
            nc.scalar.activation(rec[:], lnv[:], AF.Exp, scale=-1.0,
                                 bias=ln8_c[:])
            s2 = s3p.tile([128, GROUP], F32, tag="s2")
            nc.vector.scalar_tensor_tensor(s2[:], dot[:], 0.0, rec[:],
                                           ALU.min, ALU.mult)

            ot = outp.tile([128, 3, GROUP], F32, tag="ot")
            for i in range(3):
                mi = s3p.tile([128, GROUP], F32, tag=f"mi{i}")
                nc.gpsimd.tensor_tensor(mi[:], s2[:], dsb[:, i, :], ALU.mult)
                nc.vector.tensor_tensor(ot[:, i, :], xsb[:, i, :], mi[:],
                                        ALU.subtract)

            c0 = u * SUPER
            nc.sync.dma_start(OUT[:, :, c0:c0 + GROUP], ot[0:64])
            nc.sync.dma_start(OUT[:, :, c0 + GROUP:c0 + SUPER], ot[64:128])

    nc.compile()
    return nc


_NC = None


def _get_nc():
    global _NC
    if _NC is None:
        _NC = _build_nc()
    return _NC


def _weight_stack(Wa, Wb, Wc, Wd):
    Z = np.zeros((64, 64), np.float32)

    def blk(a, b):
        return np.block([[a, Z], [Z, b]]).astype(np.float32)

    WaT = Wa.T.astype(np.float32)
    WbT = Wb.T.astype(np.float32)
    W2nT = (Wa - Wc).T.astype(np.float32)
    W2T = (Wc - Wa).T.astype(np.float32)
    WdT = Wd.T.astype(np.float32)
    w = np.stack([
        blk(WaT, WaT),
        blk(W2nT, W2T),
        blk(WbT, WbT),
        np.block([[WaT, Z], [-WbT, Z]]).astype(np.float32),
        blk(WdT, WdT),
        np.eye(128, dtype=np.float32),
    ])
    return np.ascontiguousarray(w, np.float32)


def run_full(X, J, Wa, Wb, Wc, Wd, trace=False, trace_kwargs=None):
    nc = _get_nc()
    wmm = _weight_stack(Wa, Wb, Wc, Wd)
    in_maps = []
    for b in range(B):
        in_maps.append({
            "XS": np.ascontiguousarray(X[b].reshape(C, 192), np.float32),
            "JS": np.ascontiguousarray(J[b].reshape(C, 192), np.float32),
            "WMM": wmm,
        })
    res = bass_utils.run_bass_kernel_spmd(
        nc, in_maps, core_ids=list(range(B)), trace=trace,
        **(trace_kwargs or {}))
    out = np.stack([res.results[b]["OUT"] for b in range(B)])
    return out.astype(np.float32), res


def kernel(X, J, Wa, Wb, Wc, Wd):
    out, _ = run_full(X, J, Wa, Wb, Wc, Wd)
    return out



# revision 9
# speedup vs baseline: 1.2483x; 1.2483x over previous
"""Trainium2 Bass kernel for nn_Complex_Only_46308337385506 (gnn_message_passing).

Math (validated against the jax reference by the prior baseline):
  Per edge, the basis R enters only through two scalars:
      gam = Jz/|J|
      a1  = copysign(sqrt(Jx^2+Jy^2)/|J|, gam+eps)
  With w = gam*Xz - a1*Xx and c8 = a1*Xz + gam*Xx:
      Y0 = Wa@Xx + (Wa-Wc)@(a1*w) + Wb@(gam*Xy)
      Y1 = Wa@Xy - Wb@c8
      Y2 = Wa@Xz + (Wc-Wa)@(gam*w) + Wb@(a1*Xy)
  VN leaky relu (d = Wd@Y over channels, per (f, point)):
      dot = <Y, d>_3 ; dn2 = <d, d>_3
      out = Y - 0.8*min(dot,0)/(dn2+eps) * d
  d is computed directly from the same matmul RHS as Y using host-fused
  weights (Wd@Wk), eliminating the separate Wd stage entirely.

Implementation strategy (cost-model driven):
  - bf16 end to end: halves DMA bytes, 1-cycle/row matmuls+transposes on PE,
    2x DVE modes on packed elementwise ops. rel-err lands ~1e-2 >> margin
    under the 2e-2 gate.
  - Host repacks X/J as [point, comp, edge] partition-major rows so every
    DMA descriptor moves >=512B contiguous and component views are packed.
  - Per super (1024 points = 8 slots x 128 partitions): per-edge scalars in
    point-major layout (batched across 2 supers), PE-transpose gam/a1/X into
    bf16 PSUM ([2 slot-parities x 64 edges] on partitions), products in the
    transposed domain, 16 accumulating bf16 matmuls (Y and fused-d), VN
    stage reading Y/d PSUM banks directly.
  - Custom DVE ops fuse: q=x^2+y^2, n2=q+z^2, a1=copysign, s2=min(dot,0)*rec0^2.
  - ACT table pinned to reciprocal_sqrt_and_small (Rsqrt/Sign/Copy).
"""

import numpy as np
from contextlib import ExitStack

import concourse.bass as bass
import concourse.bacc as bacc
import concourse.tile as tile
from concourse import mybir
from concourse import bass_utils

F32 = mybir.dt.float32
BF16 = mybir.dt.bfloat16
AF = mybir.ActivationFunctionType
ALU = mybir.AluOpType

EPS = 1e-6
NEG = 0.2

B, C, E = 8, 16384, 64
SUPER = 1024           # points per super-iteration
NSUP = C // SUPER      # 16
ROW = 3 * E            # 192 values per point
NW = 11                # weight mats incl. identity

_CUSTOM_OPS = {}


def _register_custom_dve_ops():
    """Register fused DVE ops (module-level, idempotent):
      SQSUM:   out = Src0^2 + Src1^2
      ADDSQ:   out = Src0 + Src1^2
      CPSIGN:  out = Src0 >= c0 ? Src1 : -Src1      (copysign w/ eps shift)
      MINSQM:  out = min(Src0, 0) * Src1^2          (VN-relu gate)
    """
    if _CUSTOM_OPS:
        return _CUSTOM_OPS
    import numpy as _np
    from concourse import dve_ops
    from concourse.dve_spec import (
        Spec, Src0, Src1, C0, Zero, lower, sq, minn, select, _has_src1)
    from concourse.dve_uop import DveOpSpec
    from concourse.dve_table_gen import dve_ver_for

    def make(name, body, ref):
        spec = Spec(body=body, reference=ref)
        opcode = dve_ops._CUSTOM_DVE_ROW_BASE + len(dve_ops.OPS)
        shas = {}
        for ver in ("v3", "v4"):
            try:
                s = DveOpSpec(name=name, opcode=opcode,
                              uops=lower(spec, ver=ver),
                              rd1_en=_has_src1(spec))
                shas[ver] = s.sha(ver)
            except Exception:
                pass
        op = dve_ops.DveOp(name, spec, subdim=False, uops_sha=shas)
        dve_ops.OPS.append(op)
        dve_ops.CUSTOM_DVE_SPECS[name] = spec
        dve_ops._SUB_OPCODE_FOR_NAME[name] = opcode
        assert opcode < 0x20
        return op

    def flat(a):
        return a.astype(_np.float32).reshape(a.shape[0], -1)

    _CUSTOM_OPS["SQSUM"] = make(
        "SQSUM_ANT", sq(Src0) + sq(Src1),
        lambda in0, in1, s0, s1, imm2:
            flat(in0) ** 2 + flat(in1) ** 2)
    _CUSTOM_OPS["ADDSQ"] = make(
        "ADDSQ_ANT", Src0 + sq(Src1),
        lambda in0, in1, s0, s1, imm2:
            flat(in0) + flat(in1) ** 2)
    _CUSTOM_OPS["CPSIGN"] = make(
        "CPSIGN_ANT", select(Src0 >= C0, Src1, Zero - Src1),
        lambda in0, in1, s0, s1, imm2:
            _np.where(flat(in0) >= s0, flat(in1), -flat(in1)))
    _CUSTOM_OPS["MINSQM"] = make(
        "MINSQM_ANT", minn(Src0, Zero) * sq(Src1),
        lambda in0, in1, s0, s1, imm2:
            _np.minimum(flat(in0), 0.0) * flat(in1) ** 2)
    return _CUSTOM_OPS


def _pin_act_table_set(arch: str):
    """Steer the ACT table-set chooser so all funcs used here first-match
    sqrt_and_others -> exactly one table load."""
    from concourse import hw_specs
    tables = hw_specs.get_activation_tables(arch)  # cached dict, mutated
    mine = {AF.Sqrt, AF.Sign, AF.Copy, AF.Identity, AF.Square}
    for name, funcs in tables.items():
        if name != "sqrt_and_others":
            funcs -= mine


def _build_nc():
    global OPS
    OPS = _register_custom_dve_ops()
    nc = bacc.Bacc("TRN2", debug=False)
    _pin_act_table_set(nc.m.arch)

    XS = nc.dram_tensor("XS", [NSUP * 64, 16 * ROW], BF16,
                        kind="ExternalInput").ap()
    JS = nc.dram_tensor("JS", [NSUP * 64, 16 * ROW], BF16,
                        kind="ExternalInput").ap()
    WMM = nc.dram_tensor("WMM", [128, NW * 128], BF16,
                         kind="ExternalInput").ap()
    OUT = nc.dram_tensor("OUT", [NSUP * 128, 8 * ROW], BF16,
                         kind="ExternalOutput").ap()

    # DRAM views: row (u2, p) holds supers (2*u2, 2*u2+1); within a row the
    # layout is [uu:2][s:8][i:3][e:64].
    X3 = XS.rearrange("(v p) (uu w) -> v p uu w", p=128, uu=2)
    J3 = JS.rearrange("(v p) (uu w) -> v p uu w", p=128, uu=2)
    O3 = OUT.rearrange("(u p) w -> u p w", p=128)

    with tile.TileContext(nc) as tc, ExitStack() as ctx:
        const = ctx.enter_context(tc.tile_pool(name="const", bufs=1))
        io = ctx.enter_context(tc.tile_pool(name="io", bufs=2))
        sa = ctx.enter_context(tc.tile_pool(name="sa", bufs=2))
        sxp = ctx.enter_context(tc.tile_pool(name="sxp", bufs=2))
        prp = ctx.enter_context(tc.tile_pool(name="prp", bufs=2))
        s3p = ctx.enter_context(tc.tile_pool(name="s3p", bufs=2))
        outp = ctx.enter_context(tc.tile_pool(name="outp", bufs=2))
        psT = ctx.enter_context(tc.tile_pool(name="psT", bufs=1, space="PSUM"))
        psY = ctx.enter_context(tc.tile_pool(name="psY", bufs=1, space="PSUM"))
        psD = ctx.enter_context(tc.tile_pool(name="psD", bufs=1, space="PSUM"))

        # bias constants for ACT (f32 [128,1])
        b_n2 = const.tile([128, 1], F32, tag="b_n2")
        b_q = const.tile([128, 1], F32, tag="b_q")
        b_d = const.tile([128, 1], F32, tag="b_d")
        nc.gpsimd.memset(b_n2[:], 1e-12)
        nc.gpsimd.memset(b_q[:], 1e-20)
        nc.gpsimd.memset(b_d[:], 1.25 * EPS)

        # weights + identity, loaded once: [128, 11, 128]
        wsb = const.tile([128, NW, 128], BF16, tag="wsb")
        nc.sync.dma_start(wsb[:].rearrange("p n m -> p (n m)"), WMM)
        W_A = wsb[:, 0, :]
        W_2 = wsb[:, 1, :]
        W_2n = wsb[:, 2, :]
        W_B = wsb[:, 3, :]
        W_Bn = wsb[:, 4, :]
        W_DA = wsb[:, 5, :]
        W_D2 = wsb[:, 6, :]
        W_D2n = wsb[:, 7, :]
        W_DB = wsb[:, 8, :]
        W_DBn = wsb[:, 9, :]
        IDT = wsb[:, 10, :]

        for u in range(NSUP):
            if u % 2 == 0:
                xs2 = io.tile([128, 2, ROW * 8], BF16, tag="xs")
                js2 = io.tile([128, 2, ROW * 8], BF16, tag="js")
                nc.sync.dma_start(xs2[:], X3[u // 2])
                nc.sync.dma_start(js2[:], J3[u // 2])

                # ---- stage A on the 2-super pair: per-edge scalars --------
                jv = js2[:].rearrange("p uu (s i e) -> p (uu s) i e",
                                      s=8, i=3, e=64)
                q = sa.tile([128, 16, 64], BF16, tag="q")
                n2 = sa.tile([128, 16, 64], BF16, tag="n2")
                nc.vector._custom_dve(OPS["SQSUM"], out=q[:],
                                      in0=jv[:, :, 0, :], in1=jv[:, :, 1, :])
                nc.vector._custom_dve(OPS["ADDSQ"], out=n2[:],
                                      in0=q[:], in1=jv[:, :, 2, :])
                s_ = sa.tile([128, 16, 64], BF16, tag="s_")
                sq_ = sa.tile([128, 16, 64], BF16, tag="sq_")
                nc.scalar.activation(s_[:], n2[:], AF.Sqrt, bias=b_n2[:])
                nc.scalar.activation(sq_[:], q[:], AF.Sqrt, bias=b_q[:])
                t_ = sa.tile([128, 16, 64], BF16, tag="t_")
                with nc.allow_low_precision("bf16 kernel, 2e-2 gate"):
                    nc.vector.reciprocal(t_[:], s_[:])
                rho = sa.tile([128, 16, 64], BF16, tag="rho")
                gam = sa.tile([128, 16, 64], BF16, tag="gam")
                nc.vector.tensor_tensor(rho[:], sq_[:], t_[:], ALU.mult)
                nc.gpsimd.tensor_tensor(gam[:], jv[:, :, 2, :], t_[:],
                                        ALU.mult)
                a1 = sa.tile([128, 16, 64], BF16, tag="a1")
                nc.vector._custom_dve(OPS["CPSIGN"], out=a1[:],
                                      in0=gam[:], in1=rho[:], s0=-EPS)

            half = u % 2
            xv = xs2[:, half].rearrange("p (s i e) -> p s i e", s=8, i=3,
                                        e=64)
            gv = gam[:, 8 * half:8 * half + 8]     # [128, 8, 64]
            av = a1[:, 8 * half:8 * half + 8]

            # ---- transposes into bf16 PSUM ---------------------------------
            # partitions become (slot-parity h, edge e); free = (k, point p)
            Tga = psT.tile([128, 2, 512], BF16, tag="Tga")
            TX = psT.tile([128, 2, 512], BF16, tag="TX")
            for k in range(4):
                sl = slice(128 * k, 128 * (k + 1))
                nc.tensor.transpose(Tga[:, 0, sl], gv[:, 2 * k:2 * k + 2, :],
                                    IDT)
                nc.tensor.transpose(Tga[:, 1, sl], av[:, 2 * k:2 * k + 2, :],
                                    IDT)
            Tg = Tga[:, 0]
            Ta = Tga[:, 1]

            # X components through a 2-slot PSUM transit, copied to SBUF
            sx = []
            for i in range(3):
                for k in range(4):
                    sl = slice(128 * k, 128 * (k + 1))
                    nc.tensor.transpose(TX[:, i % 2, sl],
                                        xv[:, 2 * k:2 * k + 2, i, :], IDT)
                sxi = sxp.tile([128, 512], BF16, tag=f"sx{i}")
                nc.scalar.activation(sxi[:], TX[:, i % 2], AF.Copy)
                sx.append(sxi)
            sxx, sxy, sxz = sx

            # ---- products in the transposed domain -------------------------
            m1 = prp.tile([128, 512], BF16, tag="m1")
            m2 = prp.tile([128, 512], BF16, tag="m2")
            w = prp.tile([128, 512], BF16, tag="w")
            nc.vector.tensor_tensor(m1[:], Tg, sxz[:], ALU.mult)
            nc.vector.tensor_tensor(m2[:], Ta, sxx[:], ALU.mult)
            nc.vector.tensor_tensor(w[:], m1[:], m2[:], ALU.subtract)
            pw = prp.tile([128, 512], BF16, tag="pw")
            gw = prp.tile([128, 512], BF16, tag="gw")
            gy = prp.tile([128, 512], BF16, tag="gy")
            ay = prp.tile([128, 512], BF16, tag="ay")
            nc.gpsimd.tensor_tensor(pw[:], Ta, w[:], ALU.mult)
            nc.gpsimd.tensor_tensor(gw[:], Tg, w[:], ALU.mult)
            nc.gpsimd.tensor_tensor(gy[:], Tg, sxy[:], ALU.mult)
            nc.gpsimd.tensor_tensor(ay[:], Ta, sxy[:], ALU.mult)
            m3 = prp.tile([128, 512], BF16, tag="m3")
            m4 = prp.tile([128, 512], BF16, tag="m4")
            c8 = prp.tile([128, 512], BF16, tag="c8")
            nc.gpsimd.tensor_tensor(m3[:], Ta, sxz[:], ALU.mult)
            nc.gpsimd.tensor_tensor(m4[:], Tg, sxx[:], ALU.mult)
            nc.vector.tensor_tensor(c8[:], m3[:], m4[:], ALU.add)

            # ---- Y and fused-d matmuls -------------------------------------
            Y = [psY.tile([128, 512], F32, tag=f"Y{i}", name=f"Y{i}")
                 for i in range(3)]
            D = [psD.tile([128, 512], F32, tag=f"D{i}", name=f"D{i}")
                 for i in range(3)]
            for ps, WA_, W2_, WB_ in ((Y, W_A, W_2, W_B),
                                      (D, W_DA, W_D2, W_DB)):
                nc.tensor.matmul(ps[0][:], WA_, sxx[:], start=True,
                                 stop=False)
                nc.tensor.matmul(ps[0][:], W2_, pw[:], start=False,
                                 stop=False)
                nc.tensor.matmul(ps[0][:], WB_, gy[:], start=False, stop=True)
                nc.tensor.matmul(ps[1][:], WA_, sxy[:], start=True,
                                 stop=False)
            nc.tensor.matmul(Y[1][:], W_Bn, c8[:], start=False, stop=True)
            nc.tensor.matmul(D[1][:], W_DBn, c8[:], start=False, stop=True)
            for ps, WA_, W2n_, WB_ in ((Y, W_A, W_2n, W_B),
                                       (D, W_DA, W_D2n, W_DB)):
                nc.tensor.matmul(ps[2][:], WA_, sxz[:], start=True,
                                 stop=False)
                nc.tensor.matmul(ps[2][:], W2n_, gw[:], start=False,
                                 stop=False)
                nc.tensor.matmul(ps[2][:], WB_, ay[:], start=False, stop=True)

            # ---- VN leaky relu, reading Y/d straight from PSUM -------------
            xd0 = s3p.tile([128, 512], BF16, tag="xd0")
            xd1 = s3p.tile([128, 512], BF16, tag="xd1")
            xd2 = s3p.tile([128, 512], BF16, tag="xd2")
            nc.gpsimd.tensor_tensor(xd0[:], Y[0][:], D[0][:], ALU.mult)
            nc.gpsimd.tensor_tensor(xd1[:], Y[1][:], D[1][:], ALU.mult)
            nc.gpsimd.tensor_tensor(xd2[:], Y[2][:], D[2][:], ALU.mult)
            dot = s3p.tile([128, 512], BF16, tag="dot")
            nc.vector.tensor_tensor(dot[:], xd0[:], xd1[:], ALU.add)
            nc.vector.tensor_tensor(dot[:], dot[:], xd2[:], ALU.add)
            dn2 = s3p.tile([128, 512], BF16, tag="dn2")
            nc.vector._custom_dve(OPS["SQSUM"], out=dn2[:],
                                  in0=D[0][:], in1=D[1][:])
            nc.vector._custom_dve(OPS["ADDSQ"], out=dn2[:],
                                  in0=dn2[:], in1=D[2][:])
            # rec0 = sqrt(0.8)/sqrt(dn2+eps);  s2 = min(dot,0)*rec0^2
            rs = s3p.tile([128, 512], BF16, tag="rs")
            nc.scalar.activation(rs[:], dn2[:], AF.Sqrt, scale=1.25,
                                 bias=b_d[:])
            rec0 = s3p.tile([128, 512], BF16, tag="rec0")
            with nc.allow_low_precision("bf16 kernel, 2e-2 gate"):
                nc.vector.reciprocal(rec0[:], rs[:])
            s2 = s3p.tile([128, 512], BF16, tag="s2")
            nc.vector._custom_dve(OPS["MINSQM"], out=s2[:],
                                  in0=dot[:], in1=rec0[:])

            ot = outp.tile([128, 3, 512], BF16, tag="ot")
            for i in range(3):
                mi = s3p.tile([128, 512], BF16, tag=f"mi{i}")
                nc.gpsimd.tensor_tensor(mi[:], s2[:], D[i][:], ALU.mult)
                nc.gpsimd.tensor_tensor(ot[:, i, :], Y[i][:], mi[:],
                                        ALU.subtract)

            nc.sync.dma_start(O3[u], ot[:].rearrange("p i n -> p (i n)"))

    nc.compile()
    return nc


_NC = None


def _get_nc():
    global _NC
    if _NC is None:
        _NC = _build_nc()
    return _NC


def _to_bf16(a):
    import ml_dtypes
    return np.asarray(a, np.float32).astype(ml_dtypes.bfloat16)


def _pack_input(A):
    """[C, E, 3] f32 -> [NSUP*128, 2*ROW] bf16 with row (u2, p) holding
    supers (2u2, 2u2+1); per-row layout [uu][s][i][e]; point
    c = u*1024 + s*128 + p."""
    a = np.asarray(A, np.float32).reshape(NSUP, 8, 128, E, 3)
    a = a.transpose(0, 2, 1, 4, 3)          # [u, p, s, i, e]
    a = a.reshape(NSUP // 2, 2, 128, ROW * 8).transpose(0, 2, 1, 3)
    return np.ascontiguousarray(_to_bf16(a.reshape(NSUP * 64, 16 * ROW)))


def _unpack_output(o):
    """[NSUP*128, ROW] bf16 -> [64, 3, C] f32. Device row (u, q=h*64+f)
    holds [i][k][p] with c = u*1024 + (2k+h)*128 + p."""
    a = np.asarray(o, np.float32).reshape(NSUP, 2, 64, 3, 4, 128)
    a = a.transpose(2, 3, 0, 4, 1, 5)       # [f, i, u, k, h, p]
    return np.ascontiguousarray(a.reshape(64, 3, C))


def _weight_stack(Wa, Wb, Wc, Wd):
    Wa = np.asarray(Wa, np.float32)
    Wb = np.asarray(Wb, np.float32)
    Wc = np.asarray(Wc, np.float32)
    Wd = np.asarray(Wd, np.float32)
    Z = np.zeros((64, 64), np.float32)

    def blk(m):
        return np.block([[m, Z], [Z, m]]).astype(np.float32)

    W2 = Wa - Wc
    mats = [
        blk(Wa.T), blk(W2.T), blk(-W2.T), blk(Wb.T), blk(-Wb.T),
        blk((Wd @ Wa).T), blk((Wd @ W2).T), blk(-(Wd @ W2).T),
        blk((Wd @ Wb).T), blk(-(Wd @ Wb).T),
        np.eye(128, dtype=np.float32),
    ]
    w = np.stack(mats)                       # [11, 128, 128]
    w = w.transpose(1, 0, 2).reshape(128, NW * 128)
    return np.ascontiguousarray(_to_bf16(w))


def run_full(X, J, Wa, Wb, Wc, Wd, trace=False, trace_kwargs=None):
    nc = _get_nc()
    wmm = _weight_stack(Wa, Wb, Wc, Wd)
    in_maps = []
    for b in range(B):
        in_maps.append({
            "XS": _pack_input(X[b]),
            "JS": _pack_input(J[b]),
            "WMM": wmm,
        })
    res = bass_utils.run_bass_kernel_spmd(
        nc, in_maps, core_ids=list(range(B)), trace=trace,
        **(trace_kwargs or {}))
    out = np.stack([_unpack_output(res.results[b]["OUT"]) for b in range(B)])
    return out.astype(np.float32), res


def kernel(X, J, Wa, Wb, Wc, Wd):
    out, _ = run_full(X, J, Wa, Wb, Wc, Wd)
    return out


# revision 15
# speedup vs baseline: 1.6176x; 1.2959x over previous
"""Trainium2 Bass kernel for nn_Complex_Only_46308337385506 (gnn_message_passing).

Math (validated against the jax reference by the prior baseline):
  Per edge, the basis R enters only through two scalars:
      gam = Jz/|J|
      a1  = copysign(sqrt(Jx^2+Jy^2)/|J|, gam+eps)
  With w = gam*Xz - a1*Xx and c8 = a1*Xz + gam*Xx:
      Y0 = Wa@Xx + (Wa-Wc)@(a1*w) + Wb@(gam*Xy)
      Y1 = Wa@Xy - Wb@c8
      Y2 = Wa@Xz + (Wc-Wa)@(gam*w) + Wb@(a1*Xy)
  VN leaky relu (d = Wd@Y over channels, per (f, point)):
      dot = <Y, d>_3 ; dn2 = <d, d>_3
      out = Y - 0.8*min(dot,0)/(dn2+eps) * d
  d is computed directly from the same matmul RHS as Y using host-fused
  weights (Wd@Wk), eliminating the separate Wd stage entirely.

Implementation strategy (cost-model driven):
  - bf16 end to end: halves DMA bytes, 1-cycle/row matmuls+transposes on PE,
    2x DVE modes on packed elementwise ops. rel-err lands ~1e-2 >> margin
    under the 2e-2 gate.
  - Host repacks X/J as [point, comp, edge] partition-major rows so every
    DMA descriptor moves >=512B contiguous and component views are packed.
  - Per super (1024 points = 8 slots x 128 partitions): per-edge scalars in
    point-major layout (batched across 2 supers), PE-transpose gam/a1/X into
    bf16 PSUM ([2 slot-parities x 64 edges] on partitions), products in the
    transposed domain, 16 accumulating bf16 matmuls (Y and fused-d), VN
    stage reading Y/d PSUM banks directly.
  - Custom DVE ops fuse: q=x^2+y^2, n2=q+z^2, a1=copysign, s2=min(dot,0)*rec0^2.
  - ACT table pinned to reciprocal_sqrt_and_small (Rsqrt/Sign/Copy).
"""

import numpy as np
from contextlib import ExitStack

import concourse.bass as bass
import concourse.bacc as bacc
import concourse.tile as tile
from concourse import mybir
from concourse import bass_utils

F32 = mybir.dt.float32
BF16 = mybir.dt.bfloat16
AF = mybir.ActivationFunctionType
ALU = mybir.AluOpType

EPS = 1e-6
NEG = 0.2

B, C, E = 8, 16384, 64
SUPER = 1024           # points per super-iteration
NSUP = C // SUPER      # 16
ROW = 3 * E            # 192 values per point
NW = 11                # weight mats incl. identity

_CUSTOM_OPS = {}


def _register_custom_dve_ops():
    """Register fused DVE ops (module-level, idempotent):
      SQSUM:   out = Src0^2 + Src1^2
      ADDSQ:   out = Src0 + Src1^2
      CPSIGN:  out = Src0 >= c0 ? Src1 : -Src1      (copysign w/ eps shift)
      MINSQM:  out = min(Src0, 0) * Src1^2          (VN-relu gate)
    """
    if _CUSTOM_OPS:
        return _CUSTOM_OPS
    import numpy as _np
    from concourse import dve_ops
    from concourse.dve_spec import (
        Spec, Src0, Src1, C0, C1, Zero, lower, sq, minn, select, _has_src1)
    from concourse.dve_uop import DveOpSpec
    from concourse.dve_table_gen import dve_ver_for

    def make(name, body, ref):
        spec = Spec(body=body, reference=ref)
        opcode = dve_ops._CUSTOM_DVE_ROW_BASE + len(dve_ops.OPS)
        shas = {}
        for ver in ("v3", "v4"):
            try:
                s = DveOpSpec(name=name, opcode=opcode,
                              uops=lower(spec, ver=ver),
                              rd1_en=_has_src1(spec))
                shas[ver] = s.sha(ver)
            except Exception:
                pass
        op = dve_ops.DveOp(name, spec, subdim=False, uops_sha=shas)
        dve_ops.OPS.append(op)
        dve_ops.CUSTOM_DVE_SPECS[name] = spec
        dve_ops._SUB_OPCODE_FOR_NAME[name] = opcode
        assert opcode < 0x20
        return op

    def flat(a):
        return a.astype(_np.float32).reshape(a.shape[0], -1)

    _CUSTOM_OPS["SQSUM"] = make(
        "SQSUM_ANT", sq(Src0) + sq(Src1),
        lambda in0, in1, s0, s1, imm2:
            flat(in0) ** 2 + flat(in1) ** 2)
    _CUSTOM_OPS["ADDSQ"] = make(
        "ADDSQ_ANT", Src0 + sq(Src1),
        lambda in0, in1, s0, s1, imm2:
            flat(in0) + flat(in1) ** 2)
    _CUSTOM_OPS["CPSIGN"] = make(
        "CPSIGN_ANT", select(Src0 >= C0, Src1, Zero - Src1),
        lambda in0, in1, s0, s1, imm2:
            _np.where(flat(in0) >= s0, flat(in1), -flat(in1)))
    _CUSTOM_OPS["MINSQM"] = make(
        "MINSQM_ANT", minn(Src0, Zero) * sq(Src1),
        lambda in0, in1, s0, s1, imm2:
            _np.minimum(flat(in0), 0.0) * flat(in1) ** 2)
    _CUSTOM_OPS["ADDSQB"] = make(
        "ADDSQB_ANT", (Src0 + sq(Src1)) * C0 + C1,
        lambda in0, in1, s0, s1, imm2:
            (flat(in0) + flat(in1) ** 2) * s0 + s1)
    return _CUSTOM_OPS


def _pin_act_table_set(arch: str):
    """Steer the ACT table-set chooser so all funcs used here first-match
    sqrt_and_others -> exactly one table load."""
    from concourse import hw_specs
    tables = hw_specs.get_activation_tables(arch)  # cached dict, mutated
    mine = {AF.Sqrt, AF.Sign, AF.Copy, AF.Identity, AF.Square}
    for name, funcs in tables.items():
        if name != "sqrt_and_others":
            funcs -= mine


def _build_nc():
    global OPS
    OPS = _register_custom_dve_ops()
    nc = bacc.Bacc("TRN2", debug=False)
    _pin_act_table_set(nc.m.arch)

    XS = nc.dram_tensor("XS", [NSUP * 64, 16 * ROW], BF16,
                        kind="ExternalInput").ap()
    JS = nc.dram_tensor("JS", [NSUP * 64, 16 * ROW], BF16,
                        kind="ExternalInput").ap()
    WMM = nc.dram_tensor("WMM", [128, NW * 128], BF16,
                         kind="ExternalInput").ap()
    OUT = nc.dram_tensor("OUT", [NSUP * 128, 8 * ROW], BF16,
                         kind="ExternalOutput").ap()

    # DRAM views: row (u2, p) holds supers (2*u2, 2*u2+1); within a row the
    # layout is [uu:2][s:8][i:3][e:64].
    X3 = XS.rearrange("(v p) (uu w) -> v p uu w", p=128, uu=2)
    J3 = JS.rearrange("(v p) (uu w) -> v p uu w", p=128, uu=2)
    O3 = OUT.rearrange("(u p) w -> u p w", p=128)

    with tile.TileContext(nc) as tc, ExitStack() as ctx:
        const = ctx.enter_context(tc.tile_pool(name="const", bufs=1))
        io = ctx.enter_context(tc.tile_pool(name="io", bufs=2))
        sa = ctx.enter_context(tc.tile_pool(name="sa", bufs=2))
        sxp = ctx.enter_context(tc.tile_pool(name="sxp", bufs=2))
        prp = ctx.enter_context(tc.tile_pool(name="prp", bufs=2))
        s3p = ctx.enter_context(tc.tile_pool(name="s3p", bufs=2))
        outp = ctx.enter_context(tc.tile_pool(name="outp", bufs=2))
        psT = ctx.enter_context(tc.tile_pool(name="psT", bufs=1, space="PSUM"))
        psY = ctx.enter_context(tc.tile_pool(name="psY", bufs=1, space="PSUM"))
        psD = ctx.enter_context(tc.tile_pool(name="psD", bufs=1, space="PSUM"))

        # bias constants for ACT (f32 [128,1])
        b_n2 = const.tile([128, 1], F32, tag="b_n2")
        b_q = const.tile([128, 1], F32, tag="b_q")
        b_sg = const.tile([128, 1], F32, tag="b_sg")
        nc.gpsimd.memset(b_n2[:], 1e-12)
        nc.gpsimd.memset(b_q[:], 1e-20)
        nc.gpsimd.memset(b_sg[:], EPS)

        # weights + identity, loaded once: [128, 11, 128]
        wsb = const.tile([128, NW, 128], BF16, tag="wsb")
        nc.sync.dma_start(wsb[:].rearrange("p n m -> p (n m)"), WMM)
        W_A = wsb[:, 0, :]
        W_2 = wsb[:, 1, :]
        W_2n = wsb[:, 2, :]
        W_B = wsb[:, 3, :]
        W_Bn = wsb[:, 4, :]
        W_DA = wsb[:, 5, :]
        W_D2 = wsb[:, 6, :]
        W_D2n = wsb[:, 7, :]
        W_DB = wsb[:, 8, :]
        W_DBn = wsb[:, 9, :]
        IDT = wsb[:, 10, :]

        for u in range(NSUP):
            if u % 2 == 0:
                xs2 = io.tile([128, 2, ROW * 8], BF16, tag="xs")
                js2 = io.tile([128, 2, ROW * 8], BF16, tag="js")
                nc.sync.dma_start(xs2[:], X3[u // 2])
                nc.sync.dma_start(js2[:], J3[u // 2])

                # ---- stage A on the 2-super pair: per-edge scalars --------
                jv = js2[:].rearrange("p uu (s i e) -> p (uu s) i e",
                                      s=8, i=3, e=64)
                q = sa.tile([128, 16, 64], BF16, tag="q")
                n2 = sa.tile([128, 16, 64], BF16, tag="n2")
                nc.vector._custom_dve(OPS["SQSUM"], out=q[:],
                                      in0=jv[:, :, 0, :], in1=jv[:, :, 1, :])
                nc.vector._custom_dve(OPS["ADDSQ"], out=n2[:],
                                      in0=q[:], in1=jv[:, :, 2, :])
                s_ = sa.tile([128, 16, 64], BF16, tag="s_")
                sq_ = sa.tile([128, 16, 64], BF16, tag="sq_")
                nc.scalar.activation(s_[:], n2[:], AF.Sqrt, bias=b_n2[:])
                nc.scalar.activation(sq_[:], q[:], AF.Sqrt, bias=b_q[:])
                t_ = sa.tile([128, 16, 64], BF16, tag="t_")
                with nc.allow_low_precision("bf16 kernel, 2e-2 gate"):
                    nc.vector.reciprocal(t_[:], s_[:])
                rho = sa.tile([128, 16, 64], BF16, tag="rho")
                gam = sa.tile([128, 16, 64], BF16, tag="gam")
                nc.vector.tensor_tensor(rho[:], sq_[:], t_[:], ALU.mult)
                nc.gpsimd.tensor_tensor(gam[:], jv[:, :, 2, :], t_[:],
                                        ALU.mult)
                sgn = sa.tile([128, 16, 64], BF16, tag="sgn")
                nc.scalar.activation(sgn[:], gam[:], AF.Sign, bias=b_sg[:])
                a1 = sa.tile([128, 16, 64], BF16, tag="a1")
                nc.vector.tensor_tensor(a1[:], sgn[:], rho[:], ALU.mult)

            half = u % 2
            xv = xs2[:, half].rearrange("p (s i e) -> p s i e", s=8, i=3,
                                        e=64)
            gv = gam[:, 8 * half:8 * half + 8]     # [128, 8, 64]
            av = a1[:, 8 * half:8 * half + 8]

            # ---- transposes into bf16 PSUM ---------------------------------
            # partitions become (slot-parity h, edge e); free = (k, point p)
            Tga = psT.tile([128, 2, 512], BF16, tag="Tga")
            TX = psT.tile([128, 2, 512], BF16, tag="TX")
            for k in range(4):
                sl = slice(128 * k, 128 * (k + 1))
                nc.tensor.transpose(Tga[:, 0, sl], gv[:, 2 * k:2 * k + 2, :],
                                    IDT)
                nc.tensor.transpose(Tga[:, 1, sl], av[:, 2 * k:2 * k + 2, :],
                                    IDT)
            Tg = Tga[:, 0]
            Ta = Tga[:, 1]

            # X components through a 2-slot PSUM transit, copied to SBUF
            sx = []
            for i in range(3):
                for k in range(4):
                    sl = slice(128 * k, 128 * (k + 1))
                    nc.tensor.transpose(TX[:, i % 2, sl],
                                        xv[:, 2 * k:2 * k + 2, i, :], IDT)
                sxi = sxp.tile([128, 512], BF16, tag=f"sx{i}")
                nc.scalar.activation(sxi[:], TX[:, i % 2], AF.Copy)
                sx.append(sxi)
            sxx, sxy, sxz = sx

            # ---- products in the transposed domain -------------------------
            m1 = prp.tile([128, 512], BF16, tag="m1")
            m2 = prp.tile([128, 512], BF16, tag="m2")
            w = prp.tile([128, 512], BF16, tag="w")
            nc.vector.tensor_tensor(m1[:], Tg, sxz[:], ALU.mult)
            nc.vector.tensor_tensor(m2[:], Ta, sxx[:], ALU.mult)
            nc.vector.tensor_tensor(w[:], m1[:], m2[:], ALU.subtract)
            pw = prp.tile([128, 512], BF16, tag="pw")
            gw = prp.tile([128, 512], BF16, tag="gw")
            gy = prp.tile([128, 512], BF16, tag="gy")
            ay = prp.tile([128, 512], BF16, tag="ay")
            nc.gpsimd.tensor_tensor(pw[:], Ta, w[:], ALU.mult)
            nc.gpsimd.tensor_tensor(gw[:], Tg, w[:], ALU.mult)
            nc.gpsimd.tensor_tensor(gy[:], Tg, sxy[:], ALU.mult)
            nc.gpsimd.tensor_tensor(ay[:], Ta, sxy[:], ALU.mult)
            m3 = prp.tile([128, 512], BF16, tag="m3")
            m4 = prp.tile([128, 512], BF16, tag="m4")
            c8 = prp.tile([128, 512], BF16, tag="c8")
            nc.gpsimd.tensor_tensor(m3[:], Ta, sxz[:], ALU.mult)
            nc.gpsimd.tensor_tensor(m4[:], Tg, sxx[:], ALU.mult)
            nc.vector.tensor_tensor(c8[:], m3[:], m4[:], ALU.add)

            # ---- Y and fused-d matmuls (interleaved so stage-3 starts early)
            Y = [psY.tile([128, 512], F32, tag=f"Y{i}", name=f"Y{i}")
                 for i in range(3)]
            D = [psD.tile([128, 512], F32, tag=f"D{i}", name=f"D{i}")
                 for i in range(3)]
            for i, (rh0, rh1, rh2) in enumerate(
                    ((sxx, pw, gy), (sxy, c8, None), (sxz, gw, ay))):
                for ps, WA_, W2_, WB_ in ((Y, W_A, (W_2, W_Bn, W_2n)[i],
                                           W_B),
                                          (D, W_DA, (W_D2, W_DBn,
                                                     W_D2n)[i], W_DB)):
                    nc.tensor.matmul(ps[i][:], WA_, rh0[:], start=True,
                                     stop=False)
                    if rh2 is None:
                        nc.tensor.matmul(ps[i][:], W2_, rh1[:], start=False,
                                         stop=True)
                    else:
                        nc.tensor.matmul(ps[i][:], W2_, rh1[:], start=False,
                                         stop=False)
                        nc.tensor.matmul(ps[i][:], WB_, rh2[:], start=False,
                                         stop=True)

            # ---- VN leaky relu --------------------------------------------
            # Early PSUM readers: xd products, Y->SBUF copies, dn2 customs.
            # Late consumers (ot) read the SBUF copies so Y banks free early;
            # D banks free at mi (just after s2).
            xd0 = s3p.tile([128, 512], BF16, tag="xd0")
            xd1 = s3p.tile([128, 512], BF16, tag="xd1")
            xd2 = s3p.tile([128, 512], BF16, tag="xd2")
            nc.gpsimd.tensor_tensor(xd0[:], Y[0][:], D[0][:], ALU.mult)
            nc.gpsimd.tensor_tensor(xd1[:], Y[1][:], D[1][:], ALU.mult)
            nc.gpsimd.tensor_tensor(xd2[:], Y[2][:], D[2][:], ALU.mult)
            ysb = s3p.tile([128, 3, 512], BF16, tag="ysb")
            for i in range(3):
                nc.scalar.activation(ysb[:, i, :], Y[i][:], AF.Copy)
            dn2 = s3p.tile([128, 512], BF16, tag="dn2")
            nc.vector._custom_dve(OPS["SQSUM"], out=dn2[:],
                                  in0=D[0][:], in1=D[1][:])
            # dn2s = (dn2 + D2^2)*1.25 + 1.25e-6 = 1.25*(|d|^2 + eps)
            dn2s = s3p.tile([128, 512], BF16, tag="dn2s")
            nc.vector._custom_dve(OPS["ADDSQB"], out=dn2s[:],
                                  in0=dn2[:], in1=D[2][:],
                                  s0=1.25, s1=1.25 * EPS)
            dot = s3p.tile([128, 512], BF16, tag="dot")
            nc.vector.tensor_tensor(dot[:], xd0[:], xd1[:], ALU.add)
            nc.vector.tensor_tensor(dot[:], dot[:], xd2[:], ALU.add)
            # rcp = 0.8/(|d|^2+eps);  s2 = min(dot,0)*rcp
            rcp = s3p.tile([128, 512], BF16, tag="rcp")
            with nc.allow_low_precision("bf16 kernel, 2e-2 gate"):
                nc.vector.reciprocal(rcp[:], dn2s[:])
            s2 = s3p.tile([128, 512], BF16, tag="s2")
            nc.gpsimd.scalar_tensor_tensor(s2[:], dot[:], 0.0, rcp[:],
                                           ALU.min, ALU.mult)

            ot = outp.tile([128, 3, 512], BF16, tag="ot")
            for i in range(3):
                mi = s3p.tile([128, 512], BF16, tag=f"mi{i}")
                nc.gpsimd.tensor_tensor(mi[:], s2[:], D[i][:], ALU.mult)
                nc.vector.tensor_tensor(ot[:, i, :], ysb[:, i, :], mi[:],
                                        ALU.subtract)

            nc.sync.dma_start(O3[u], ot[:].rearrange("p i n -> p (i n)"))

    nc.compile()
    return nc


_NC = None


def _get_nc():
    global _NC
    if _NC is None:
        _NC = _build_nc()
    return _NC


def _to_bf16(a):
    import ml_dtypes
    return np.asarray(a, np.float32).astype(ml_dtypes.bfloat16)


def _pack_input(A):
    """[C, E, 3] f32 -> [NSUP*128, 2*ROW] bf16 with row (u2, p) holding
    supers (2u2, 2u2+1); per-row layout [uu][s][i][e]; point
    c = u*1024 + s*128 + p."""
    a = np.asarray(A, np.float32).reshape(NSUP, 8, 128, E, 3)
    a = a.transpose(0, 2, 1, 4, 3)          # [u, p, s, i, e]
    a = a.reshape(NSUP // 2, 2, 128, ROW * 8).transpose(0, 2, 1, 3)
    return np.ascontiguousarray(_to_bf16(a.reshape(NSUP * 64, 16 * ROW)))


def _unpack_output(o):
    """[NSUP*128, ROW] bf16 -> [64, 3, C] f32. Device row (u, q=h*64+f)
    holds [i][k][p] with c = u*1024 + (2k+h)*128 + p."""
    a = np.asarray(o, np.float32).reshape(NSUP, 2, 64, 3, 4, 128)
    a = a.transpose(2, 3, 0, 4, 1, 5)       # [f, i, u, k, h, p]
    return np.ascontiguousarray(a.reshape(64, 3, C))


def _weight_stack(Wa, Wb, Wc, Wd):
    Wa = np.asarray(Wa, np.float32)
    Wb = np.asarray(Wb, np.float32)
    Wc = np.asarray(Wc, np.float32)
    Wd = np.asarray(Wd, np.float32)
    Z = np.zeros((64, 64), np.float32)

    def blk(m):
        return np.block([[m, Z], [Z, m]]).astype(np.float32)

    W2 = Wa - Wc
    mats = [
        blk(Wa.T), blk(W2.T), blk(-W2.T), blk(Wb.T), blk(-Wb.T),
        blk((Wd @ Wa).T), blk((Wd @ W2).T), blk(-(Wd @ W2).T),
        blk((Wd @ Wb).T), blk(-(Wd @ Wb).T),
        np.eye(128, dtype=np.float32),
    ]
    w = np.stack(mats)                       # [11, 128, 128]
    w = w.transpose(1, 0, 2).reshape(128, NW * 128)
    return np.ascontiguousarray(_to_bf16(w))


def run_full(X, J, Wa, Wb, Wc, Wd, trace=False, trace_kwargs=None):
    nc = _get_nc()
    wmm = _weight_stack(Wa, Wb, Wc, Wd)
    in_maps = []
    for b in range(B):
        in_maps.append({
            "XS": _pack_input(X[b]),
            "JS": _pack_input(J[b]),
            "WMM": wmm,
        })
    res = bass_utils.run_bass_kernel_spmd(
        nc, in_maps, core_ids=list(range(B)), trace=trace,
        **(trace_kwargs or {}))
    out = np.stack([_unpack_output(res.results[b]["OUT"]) for b in range(B)])
    return out.astype(np.float32), res


def kernel(X, J, Wa, Wb, Wc, Wd):
    out, _ = run_full(X, J, Wa, Wb, Wc, Wd)
    return out
